# revision 6
# baseline (speedup 1.0000x reference)
"""Trainium2 Bass kernel for GQA attention with RoPE, causal mask, and
attention sinks (nn_Attention_65094524338392).

Sharding: tensor-parallel by heads across 8 NeuronCores. Core c owns query
heads 4c..4c+3 and kv-head c (NREP=4, so kv-head groups stay aligned with
their query heads). Each core computes QKV projections over the full
sequence for its heads, flash-style causal attention, then an AllToAll
redistributes attention outputs from head-sharding to sequence-sharding so
each core computes the output projection for its 256-row sequence slice.

Math note: the sink scaling folds into the softmax normalizer:
    out = (sum_k exp(s_k) v_k) * sigmoid(lse - sink) / sum_k exp(s_k)
        = (sum_k exp(s_k) v_k) / (sum_k exp(s_k) + exp(sink))
so no logs/sigmoids are needed on device, and because |s| <= ~40 no
max-subtraction is needed for exp stability in fp32 accumulation.
"""

import os
import sys

sys.path.insert(0, "/opt/trn_rl_repo")

import ml_dtypes
import numpy as np

import concourse.bass as bass
import concourse.mybir as mybir
import concourse.tile as tile
from concourse import bacc
from concourse.bass_utils import run_bass_kernel_spmd

# Problem shapes
B, S, DIM = 1, 2048, 2048
NH, NKV, HD = 32, 8, 64
NREP = NH // NKV
SCALE = 1.0 / float(np.sqrt(HD))
NCORES = 8
HPC = NH // NCORES            # query heads per core (4)
QKV = HPC * HD + 2 * HD       # fused qkv output dim per core (384)
QW = HPC * HD                 # query width per core (256)
SB = 512                      # seq block (attention q-block)
NSB = S // SB                 # 4
NT = S // 128                 # 16 seq tiles
ND = DIM // 128               # 16 contraction tiles
MYS = S // NCORES             # output rows per core (256)

F32 = mybir.dt.float32
BF16 = mybir.dt.bfloat16

_cache = {}

last_exec_time_ns = None


def _install_ntff_shim():
    """Register the NTFF profile hook so trace=True yields exec_time_ns."""
    import types
    if "antenv.axon_hooks" in sys.modules:
        return
    import antenv
    mod = types.ModuleType("antenv.axon_hooks")
    mod._hook = None
    mod.set_axon_ntff_profile_hook = lambda h: setattr(mod, "_hook", h)
    mod.get_axon_ntff_profile_hook = lambda: mod._hook
    sys.modules["antenv.axon_hooks"] = mod
    antenv.axon_hooks = mod
    from trn_agent_boot.trn_boot import _ntff_profile_via_ctypes
    hook = _ntff_profile_via_ctypes("/opt/axon/libaxon_pjrt.so")
    if hook is not None:
        mod._hook = hook


def _build():
    nc = bacc.Bacc("TRN2", target_bir_lowering=False, debug=False,
                   num_devices=NCORES)

    x_e = nc.declare_dram_parameter("x", [S, DIM], F32, isOutput=False)
    wqkvT_e = nc.declare_dram_parameter("wqkvT", [DIM, QKV], BF16, isOutput=False)
    qkvb_e = nc.declare_dram_parameter("qkvb", [1, QKV], F32, isOutput=False)
    cosd_e = nc.declare_dram_parameter("cosd", [S, HD], F32, isOutput=False)
    nsin_e = nc.declare_dram_parameter("nsin", [S, HD // 2], F32, isOutput=False)
    psin_e = nc.declare_dram_parameter("psin", [S, HD // 2], F32, isOutput=False)
    mask_e = nc.declare_dram_parameter("masks", [4, 128, SB], BF16, isOutput=False)
    ident_e = nc.declare_dram_parameter("ident", [128, 128], BF16, isOutput=False)
    woT_e = nc.declare_dram_parameter("woT", [NH * HD, DIM], BF16, isOutput=False)
    wob_e = nc.declare_dram_parameter("wob", [1, DIM], F32, isOutput=False)
    sinks_e = nc.declare_dram_parameter("sinks4", [1, HPC], F32, isOutput=False)
    out_e = nc.declare_dram_parameter("out", [MYS, DIM], F32, isOutput=True)

    with tile.TileContext(nc) as tc:
        with tc.tile_pool(name="const", bufs=1) as cp, \
             tc.tile_pool(name="xstage", bufs=2) as xsp, \
             tc.tile_pool(name="xT", bufs=2) as xtp, \
             tc.tile_pool(name="rope", bufs=2) as rp, \
             tc.tile_pool(name="qkr", bufs=5) as qkrp, \
             tc.tile_pool(name="pt", bufs=4) as ptp, \
             tc.tile_pool(name="ep", bufs=2) as epp, \
             tc.tile_pool(name="fin", bufs=2) as fnp, \
             tc.tile_pool(name="pp", bufs=2, space="PSUM") as pp, \
             tc.tile_pool(name="dram", bufs=1, space="DRAM") as dp:

            # ---- constants ---- (small/startup-critical first; sync
            # queue is FIFO so the 8MB woT load goes last)
            ident_sb = cp.tile([128, 128], BF16)
            nc.sync.dma_start(ident_sb[:], ident_e[:])
            qkvb_sb = cp.tile([1, QKV], F32)
            nc.sync.dma_start(qkvb_sb[:], qkvb_e[:])
            cos_sb = cp.tile([128, NT, HD], F32)
            nc.sync.dma_start(cos_sb[:], cosd_e[:].rearrange(
                "(o p) f -> p o f", p=128))
            nsin_sb = cp.tile([128, NT, HD // 2], F32)
            nc.sync.dma_start(nsin_sb[:], nsin_e[:].rearrange(
                "(o p) f -> p o f", p=128))
            psin_sb = cp.tile([128, NT, HD // 2], F32)
            nc.sync.dma_start(psin_sb[:], psin_e[:].rearrange(
                "(o p) f -> p o f", p=128))
            sinks_sb = cp.tile([1, HPC], F32)
            nc.sync.dma_start(sinks_sb[:], sinks_e[:])
            es_sb = cp.tile([1, HPC], F32)
            nc.scalar.activation(es_sb[:], sinks_sb[:],
                                 mybir.ActivationFunctionType.Exp)
            wqkvT_sb = cp.tile([128, ND, QKV], BF16)
            nc.sync.dma_start(wqkvT_sb[:], wqkvT_e[:].rearrange(
                "(o p) f -> p o f", p=128))
            mask_sb = cp.tile([128, 4, SB], BF16)
            nc.sync.dma_start(mask_sb[:], mask_e[:].rearrange(
                "d p f -> p d f"))
            wob_sb = cp.tile([1, DIM], F32)
            nc.sync.dma_start(wob_sb[:], wob_e[:])
            woT_sb = cp.tile([128, ND, DIM], BF16)
            nc.sync.dma_start(woT_sb[:], woT_e[:].rearrange(
                "(o p) f -> p o f", p=128))
            ones_sb = cp.tile([1, 128], F32)
            nc.gpsimd.memset(ones_sb[:], 1.0)

            # persistent activations
            qT = [cp.tile([HD, S], BF16, name=f"qT{h}") for h in range(HPC)]
            kT = cp.tile([HD, S], BF16)
            v_sb = cp.tile([128, NT, HD + 1], BF16)
            nc.gpsimd.memset(v_sb[:, :, HD:HD + 1], 1.0)
            oT = [cp.tile([HD, S], BF16, name=f"oT{h}") for h in range(HPC)]

            a2a_in = dp.tile([S, MYS], BF16)
            a2a_out = dp.tile([S, MYS], BF16)

            # ---- main loop over seq blocks ----
            for s in range(NSB):
                qkr_tiles = []
                for tt in range(4):
                    t = 4 * s + tt
                    # stage x rows (cast f32 -> bf16 during DMA)
                    xst = xsp.tile([128, DIM], BF16, tag="xst")
                    nc.gpsimd.dma_start(
                        xst[:], x_e[t * 128:(t + 1) * 128, :])
                    # transpose x tile: xT_t [128(dim), ND, 128(seq)]
                    xT_t = xtp.tile([128, ND, 128], BF16, tag="xT")
                    for dq in range(ND // 4):
                        tp_ps = pp.tile([128, 512], BF16, tag="tp", bufs=2)
                        for c in range(4):
                            d = 4 * dq + c
                            nc.tensor.transpose(
                                tp_ps[:, c * 128:(c + 1) * 128],
                                xst[:, d * 128:(d + 1) * 128], ident_sb[:])
                        if dq % 2 == 0:
                            nc.scalar.copy(xT_t[:, 4 * dq:4 * dq + 4, :], tp_ps[:])
                        else:
                            nc.vector.tensor_copy(xT_t[:, 4 * dq:4 * dq + 4, :], tp_ps[:])

                    # fused qkv projection for this seq tile
                    acc = pp.tile([128, QKV], F32, tag="acc", bufs=2)
                    for d in range(ND):
                        nc.tensor.matmul(acc[:], xT_t[:, d, :],
                                         wqkvT_sb[:, d, :],
                                         start=(d == 0), stop=False)
                    nc.tensor.matmul(acc[:], ones_sb[0:1, :], qkvb_sb[:],
                                     start=False, stop=True)

                    # rope on q and k halves (free-dim ops, 5 = 4q + 1k groups)
                    W = QW + HD  # 320
                    tmp = rp.tile([128, W], F32, tag="tmp")
                    tcs = rp.tile([128, W], F32, tag="tcs")
                    qkr = qkrp.tile([128, W], BF16, tag="qkr")
                    acc5 = acc[:, 0:W].rearrange("p (h x) -> p h x", x=HD)
                    tmp5 = tmp[:].rearrange("p (h x) -> p h x", x=HD)
                    nc.vector.tensor_tensor(
                        tmp5[:, :, 0:HD // 2], acc5[:, :, HD // 2:HD],
                        nsin_sb[:, t:t + 1, :].to_broadcast([128, 5, HD // 2]),
                        mybir.AluOpType.mult)
                    nc.vector.tensor_tensor(
                        tmp5[:, :, HD // 2:HD], acc5[:, :, 0:HD // 2],
                        psin_sb[:, t:t + 1, :].to_broadcast([128, 5, HD // 2]),
                        mybir.AluOpType.mult)
                    nc.vector.tensor_tensor(
                        tcs[:], acc[:, 0:W],
                        cos_sb[:, t:t + 1, :].to_broadcast([128, 5, HD]),
                        mybir.AluOpType.mult)
                    nc.vector.tensor_tensor(qkr[:], tcs[:], tmp[:],
                                            mybir.AluOpType.add)
                    qkr_tiles.append(qkr)
                    # v: plain copy (cast to bf16)
                    nc.scalar.copy(v_sb[:, t, 0:HD], acc[:, QW + HD:QKV])

                # transpose rope'd q/k for the block into [hd, seq] layout
                for h in range(HPC + 1):
                    tq_ps = pp.tile([HD, 512], BF16, tag="tp", bufs=2)
                    for tt in range(4):
                        nc.tensor.transpose(
                            tq_ps[:, tt * 128:(tt + 1) * 128],
                            qkr_tiles[tt][:, h * HD:(h + 1) * HD], ident_sb[:])
                    dst = qT[h] if h < HPC else kT
                    if h % 2 == 0:
                        nc.scalar.copy(dst[:, s * SB:(s + 1) * SB], tq_ps[:])
                    else:
                        nc.vector.tensor_copy(dst[:, s * SB:(s + 1) * SB], tq_ps[:])

                # ---- attention for q-block j = s, all 4 heads ----
                n_kt = 4 * (s + 1)
                for h in range(HPC):
                    qs = qT[h][:, s * SB:(s + 1) * SB]
                    pv = pp.tile([HD + 1, 512], F32, tag="pv", bufs=2)
                    pts = []
                    for p in range(n_kt // 2):
                        for half in range(2):
                            i = 2 * p + half
                            sc = pp.tile([128, 512], F32, tag="sc", bufs=2)
                            nc.tensor.matmul(
                                sc[:], kT[:, i * 128:(i + 1) * 128], qs,
                                start=True, stop=True)
                            pt = ptp.tile([128, 512], BF16, tag="pt")
                            nc.scalar.activation(
                                pt[:], sc[:],
                                mybir.ActivationFunctionType.Exp, scale=SCALE)
                            if i >= 4 * s:  # diagonal tile: causal mask
                                nc.vector.tensor_tensor(
                                    pt[:], pt[:], mask_sb[:, i - 4 * s, :],
                                    mybir.AluOpType.mult)
                            pts.append(pt)
                        if p > 0:
                            for half in range(2):
                                i = 2 * (p - 1) + half
                                nc.tensor.matmul(
                                    pv[:], v_sb[:, i, :], pts[i][:],
                                    start=(i == 0), stop=False)
                    for half in range(2):
                        i = n_kt - 2 + half
                        nc.tensor.matmul(pv[:], v_sb[:, i, :], pts[i][:],
                                         start=(i == 0), stop=(i == n_kt - 1))

                    # epilogue: out_h = pv[0:64] / (S_row + exp(sink_h))
                    srow = epp.tile([1, 512], F32, tag="srow", bufs=1)
                    nc.vector.tensor_copy(srow[:], pv[HD:HD + 1, :])
                    nc.vector.tensor_scalar(srow[:], srow[:],
                                            es_sb[0:1, h:h + 1], None,
                                            mybir.AluOpType.add)
                    rrow = epp.tile([1, 512], F32, tag="rrow", bufs=1)
                    nc.vector.reciprocal(rrow[:], srow[:])
                    rbc = epp.tile([HD, 512], F32, tag="rbc")
                    nc.gpsimd.partition_broadcast(rbc[:], rrow[0:1, :])
                    nc.vector.tensor_tensor(
                        oT[h][:, s * SB:(s + 1) * SB], pv[0:HD, :], rbc[:],
                        mybir.AluOpType.mult)
                    # stream this (head, block) slice into its A2A shards
                    nc.sync.dma_start(
                        a2a_in[:].rearrange("(j hh p) n -> hh p j n",
                                            j=NCORES, hh=HPC)[h][:, 2 * s:2 * s + 2],
                        oT[h][:, s * SB:(s + 1) * SB].rearrange(
                            "p (j n) -> p j n", j=2))

            # ---- redistribute heads -> sequence slices (AllToAll) ----
            nc.gpsimd.collective_compute(
                "AllToAll", mybir.AluOpType.bypass,
                replica_groups=[list(range(NCORES))],
                ins=[a2a_in.opt()], outs=[a2a_out.opt()])
            ag_sb = cp.tile([128, ND, MYS], BF16)
            nc.sync.dma_start(ag_sb[:], a2a_out[:].rearrange(
                "(o p) n -> p o n", p=128))

            # ---- output projection for my sequence slice ----
            for m in range(MYS // 128):
                for n in range(DIM // 512):
                    fp = pp.tile([128, 512], F32, tag="acc", bufs=2)
                    for kt in range(ND):
                        nc.tensor.matmul(
                            fp[:], ag_sb[:, kt, m * 128:(m + 1) * 128],
                            woT_sb[:, kt, n * 512:(n + 1) * 512],
                            start=(kt == 0), stop=False)
                    nc.tensor.matmul(fp[:], ones_sb[0:1, :],
                                     wob_sb[0:1, n * 512:(n + 1) * 512],
                                     start=False, stop=True)
                    fo = fnp.tile([128, 512], F32, tag="fo")
                    if (m * 4 + n) % 2 == 0:
                        nc.scalar.copy(fo[:], fp[:])
                    else:
                        nc.vector.tensor_copy(fo[:], fp[:])
                    nc.sync.dma_start(
                        out_e[m * 128:(m + 1) * 128,
                              n * 512:(n + 1) * 512], fo[:])

    nc.compile()
    return nc


def _host_prep(x, rope_cache, wq_w, wq_b, wk_w, wk_b, wv_w, wv_b,
               wo_w, wo_b, sinks):
    """Build the per-core input maps (sharding + layout prep)."""
    x2 = np.ascontiguousarray(np.asarray(x, np.float32).reshape(S, DIM))
    cos = np.asarray(rope_cache[:, :HD // 2], np.float32)
    sin = np.asarray(rope_cache[:, HD // 2:], np.float32)
    cosd = np.ascontiguousarray(np.concatenate([cos, cos], axis=1))
    nsin = np.ascontiguousarray(-sin)
    psin = np.ascontiguousarray(sin)
    # causal masks for the 4 diagonal 128-row k-tiles of a 512-col q block
    masks = np.zeros((4, 128, SB), np.float32)
    for d in range(4):
        for p in range(128):
            masks[d, p, d * 128 + p:] = 1.0
    masks = masks.astype(ml_dtypes.bfloat16)
    ident = np.eye(128, dtype=ml_dtypes.bfloat16)
    woT = np.ascontiguousarray(
        np.asarray(wo_w, np.float32).T.astype(ml_dtypes.bfloat16))
    wob = np.asarray(wo_b, np.float32).reshape(1, DIM)

    in_maps = []
    for c in range(NCORES):
        qsl = slice(c * QW, (c + 1) * QW)
        ksl = slice(c * HD, (c + 1) * HD)
        wqkvT = np.ascontiguousarray(np.concatenate([
            np.asarray(wq_w, np.float32)[qsl].T,
            np.asarray(wk_w, np.float32)[ksl].T,
            np.asarray(wv_w, np.float32)[ksl].T],
            axis=1).astype(ml_dtypes.bfloat16))
        qkvb = np.ascontiguousarray(np.concatenate([
            np.asarray(wq_b, np.float32)[qsl],
            np.asarray(wk_b, np.float32)[ksl],
            np.asarray(wv_b, np.float32)[ksl]])).reshape(1, QKV)
        sinks4 = np.ascontiguousarray(
            np.asarray(sinks, np.float32)[c * HPC:(c + 1) * HPC]).reshape(1, HPC)
        in_maps.append({
            "x": x2, "wqkvT": wqkvT, "qkvb": qkvb, "cosd": cosd,
            "nsin": nsin, "psin": psin, "masks": masks, "ident": ident,
            "woT": woT, "wob": wob, "sinks4": sinks4,
        })
    return in_maps


def kernel(**inputs):
    global last_exec_time_ns
    if "nc" not in _cache:
        _cache["nc"] = _build()
    nc = _cache["nc"]
    in_maps = _host_prep(**inputs)
    trace = bool(int(os.environ.get("BASS_KERNEL_TRACE", "0")))
    if trace:
        try:
            _install_ntff_shim()
        except Exception:
            trace = False
    res = run_bass_kernel_spmd(nc, in_maps, core_ids=list(range(NCORES)),
                               trace=trace)
    last_exec_time_ns = res.exec_time_ns
    out = np.concatenate([res.results[c]["out"] for c in range(NCORES)],
                         axis=0)
    return out.reshape(B, S, NH * HD)


# revision 8
# speedup vs baseline: 1.2321x; 1.2321x over previous
"""Trainium2 Bass kernel for GQA attention with RoPE, causal mask, and
attention sinks (nn_Attention_65094524338392).

Sharding: tensor-parallel by heads across 8 NeuronCores. Core c owns query
heads 4c..4c+3 and kv-head c (NREP=4, so kv-head groups stay aligned with
their query heads). Each core computes QKV projections over the full
sequence for its heads, flash-style causal attention, then an AllToAll
redistributes attention outputs from head-sharding to sequence-sharding so
each core computes the output projection for its 256-row sequence slice.

Math note: the sink scaling folds into the softmax normalizer:
    out = (sum_k exp(s_k) v_k) * sigmoid(lse - sink) / sum_k exp(s_k)
        = (sum_k exp(s_k) v_k) / (sum_k exp(s_k) + exp(sink))
so no logs/sigmoids are needed on device, and because |s| <= ~40 no
max-subtraction is needed for exp stability in fp32 accumulation.
"""

import os
import sys

sys.path.insert(0, "/opt/trn_rl_repo")

import ml_dtypes
import numpy as np

import concourse.bass as bass
import concourse.mybir as mybir
import concourse.tile as tile
from concourse import bacc
from concourse.bass_utils import run_bass_kernel_spmd

# Problem shapes
B, S, DIM = 1, 2048, 2048
NH, NKV, HD = 32, 8, 64
NREP = NH // NKV
SCALE = 1.0 / float(np.sqrt(HD))
NCORES = 8
HPC = NH // NCORES            # query heads per core (4)
QKV = HPC * HD + 2 * HD       # fused qkv output dim per core (384)
QW = HPC * HD                 # query width per core (256)
SB = 512                      # seq block (attention q-block)
NSB = S // SB                 # 4
NT = S // 128                 # 16 seq tiles
ND = DIM // 128               # 16 contraction tiles
MYS = S // NCORES             # output rows per core (256)

F32 = mybir.dt.float32
BF16 = mybir.dt.bfloat16

_cache = {}

last_exec_time_ns = None


def _install_ntff_shim():
    """Register the NTFF profile hook so trace=True yields exec_time_ns."""
    import types
    if "antenv.axon_hooks" in sys.modules:
        return
    import antenv
    mod = types.ModuleType("antenv.axon_hooks")
    mod._hook = None
    mod.set_axon_ntff_profile_hook = lambda h: setattr(mod, "_hook", h)
    mod.get_axon_ntff_profile_hook = lambda: mod._hook
    sys.modules["antenv.axon_hooks"] = mod
    antenv.axon_hooks = mod
    from trn_agent_boot.trn_boot import _ntff_profile_via_ctypes
    hook = _ntff_profile_via_ctypes("/opt/axon/libaxon_pjrt.so")
    if hook is not None:
        mod._hook = hook


def _build():
    nc = bacc.Bacc("TRN2", target_bir_lowering=False, debug=False,
                   num_devices=NCORES)

    x_e = nc.declare_dram_parameter("x", [S, DIM], F32, isOutput=False)
    wqkvT_e = nc.declare_dram_parameter("wqkvT", [DIM, QKV], BF16, isOutput=False)
    qkvb_e = nc.declare_dram_parameter("qkvb", [1, QKV], F32, isOutput=False)
    cosd_e = nc.declare_dram_parameter("cosd", [S, HD], F32, isOutput=False)
    nsin_e = nc.declare_dram_parameter("nsin", [S, HD // 2], F32, isOutput=False)
    psin_e = nc.declare_dram_parameter("psin", [S, HD // 2], F32, isOutput=False)
    mask_e = nc.declare_dram_parameter("masks", [4, 128, SB], BF16, isOutput=False)
    ident_e = nc.declare_dram_parameter("ident", [128, 128], BF16, isOutput=False)
    woT_e = nc.declare_dram_parameter("woT", [NH * HD, DIM], BF16, isOutput=False)
    wob_e = nc.declare_dram_parameter("wob", [1, DIM], F32, isOutput=False)
    sinks_e = nc.declare_dram_parameter("sinks4", [1, HPC], F32, isOutput=False)
    out_e = nc.declare_dram_parameter("out", [MYS, DIM], F32, isOutput=True)

    with tile.TileContext(nc) as tc:
        with tc.tile_pool(name="const", bufs=1) as cp, \
             tc.tile_pool(name="xstage", bufs=2) as xsp, \
             tc.tile_pool(name="xT", bufs=2) as xtp, \
             tc.tile_pool(name="rope", bufs=2) as rp, \
             tc.tile_pool(name="qkr", bufs=5) as qkrp, \
             tc.tile_pool(name="pt", bufs=4) as ptp, \
             tc.tile_pool(name="ep", bufs=2) as epp, \
             tc.tile_pool(name="fin", bufs=2) as fnp, \
             tc.tile_pool(name="dram", bufs=1, space="DRAM") as dp:

            # ---- constants ---- (small/startup-critical first; sync
            # queue is FIFO so the 8MB woT load goes last)
            ident_sb = cp.tile([128, 128], BF16)
            nc.sync.dma_start(ident_sb[:], ident_e[:])
            qkvb_sb = cp.tile([1, QKV], F32)
            nc.sync.dma_start(qkvb_sb[:], qkvb_e[:])
            cos_sb = cp.tile([128, NT, HD], F32)
            nc.sync.dma_start(cos_sb[:], cosd_e[:].rearrange(
                "(o p) f -> p o f", p=128))
            nsin_sb = cp.tile([128, NT, HD // 2], F32)
            nc.sync.dma_start(nsin_sb[:], nsin_e[:].rearrange(
                "(o p) f -> p o f", p=128))
            psin_sb = cp.tile([128, NT, HD // 2], F32)
            nc.sync.dma_start(psin_sb[:], psin_e[:].rearrange(
                "(o p) f -> p o f", p=128))
            sinks_sb = cp.tile([1, HPC], F32)
            nc.sync.dma_start(sinks_sb[:], sinks_e[:])
            es_sb = cp.tile([1, HPC], F32)
            nc.scalar.activation(es_sb[:], sinks_sb[:],
                                 mybir.ActivationFunctionType.Exp)
            wqkvT_sb = cp.tile([128, ND, QKV], BF16)
            nc.sync.dma_start(wqkvT_sb[:], wqkvT_e[:].rearrange(
                "(o p) f -> p o f", p=128))
            mask_sb = cp.tile([128, 4, SB], BF16)
            nc.sync.dma_start(mask_sb[:], mask_e[:].rearrange(
                "d p f -> p d f"))
            wob_sb = cp.tile([1, DIM], F32)
            nc.sync.dma_start(wob_sb[:], wob_e[:])
            woT_sb = cp.tile([128, ND, DIM], BF16)
            nc.sync.dma_start(woT_sb[:], woT_e[:].rearrange(
                "(o p) f -> p o f", p=128))
            ones_sb = cp.tile([1, 128], F32)
            nc.gpsimd.memset(ones_sb[:], 1.0)

            # persistent activations
            qT = [cp.tile([HD, S], BF16, name=f"qT{h}") for h in range(HPC)]
            kT = cp.tile([HD, S], BF16)
            v_sb = cp.tile([128, NT, HD + 1], BF16)
            nc.gpsimd.memset(v_sb[:, :, HD:HD + 1], 1.0)
            oT = [cp.tile([HD, S], BF16, name=f"oT{h}") for h in range(HPC)]

            a2a_in = dp.tile([S, MYS], BF16)
            a2a_out = dp.tile([S, MYS], BF16)

            # ---- phase B: QKV projections + rope + transposes ----
            with tc.tile_pool(name="ppB", bufs=2, space="PSUM") as ppB:
                for s in range(NSB):
                    qkr_tiles = []
                    for tt in range(4):
                        t = 4 * s + tt
                        # stage x rows (cast f32 -> bf16 during DMA)
                        xst = xsp.tile([128, DIM], BF16, tag="xst")
                        nc.gpsimd.dma_start(
                            xst[:], x_e[t * 128:(t + 1) * 128, :])
                        # transpose x tile: xT_t [128(dim), ND, 128(seq)]
                        xT_t = xtp.tile([128, ND, 128], BF16, tag="xT")
                        for dq in range(ND // 8):
                            tp_ps = ppB.tile([128, 1024], BF16, tag="tp", bufs=2)
                            for c in range(8):
                                d = 8 * dq + c
                                nc.tensor.transpose(
                                    tp_ps[:, c * 128:(c + 1) * 128],
                                    xst[:, d * 128:(d + 1) * 128], ident_sb[:])
                            if dq % 2 == 0:
                                nc.scalar.copy(xT_t[:, 8 * dq:8 * dq + 8, :], tp_ps[:])
                            else:
                                nc.vector.tensor_copy(xT_t[:, 8 * dq:8 * dq + 8, :], tp_ps[:])

                        # fused qkv projection for this seq tile
                        acc = ppB.tile([128, QKV], F32, tag="acc", bufs=2)
                        for d in range(ND):
                            nc.tensor.matmul(acc[:], xT_t[:, d, :],
                                             wqkvT_sb[:, d, :],
                                             start=(d == 0), stop=False)
                        nc.tensor.matmul(acc[:], ones_sb[0:1, :], qkvb_sb[:],
                                         start=False, stop=True)

                        # rope on q and k halves (free-dim ops, 5 = 4q + 1k groups)
                        W = QW + HD  # 320
                        tmp = rp.tile([128, W], F32, tag="tmp")
                        tcs = rp.tile([128, W], F32, tag="tcs")
                        qkr = qkrp.tile([128, W], BF16, tag="qkr")
                        acc5 = acc[:, 0:W].rearrange("p (h x) -> p h x", x=HD)
                        tmp5 = tmp[:].rearrange("p (h x) -> p h x", x=HD)
                        nc.vector.tensor_tensor(
                            tmp5[:, :, 0:HD // 2], acc5[:, :, HD // 2:HD],
                            nsin_sb[:, t:t + 1, :].to_broadcast([128, 5, HD // 2]),
                            mybir.AluOpType.mult)
                        nc.vector.tensor_tensor(
                            tmp5[:, :, HD // 2:HD], acc5[:, :, 0:HD // 2],
                            psin_sb[:, t:t + 1, :].to_broadcast([128, 5, HD // 2]),
                            mybir.AluOpType.mult)
                        nc.vector.tensor_tensor(
                            tcs[:], acc[:, 0:W],
                            cos_sb[:, t:t + 1, :].to_broadcast([128, 5, HD]),
                            mybir.AluOpType.mult)
                        nc.vector.tensor_tensor(qkr[:], tcs[:], tmp[:],
                                                mybir.AluOpType.add)
                        qkr_tiles.append(qkr)
                        # v: plain copy (cast to bf16)
                        nc.scalar.copy(v_sb[:, t, 0:HD], acc[:, QW + HD:QKV])

                    # transpose rope'd q/k for the block into [hd, seq] layout
                    for h in range(HPC + 1):
                        tq_ps = ppB.tile([HD, 512], BF16, tag="tq", bufs=2)
                        for tt in range(4):
                            nc.tensor.transpose(
                                tq_ps[:, tt * 128:(tt + 1) * 128],
                                qkr_tiles[tt][:, h * HD:(h + 1) * HD], ident_sb[:])
                        dst = qT[h] if h < HPC else kT
                        if h % 2 == 0:
                            nc.scalar.copy(dst[:, s * SB:(s + 1) * SB], tq_ps[:])
                        else:
                            nc.vector.tensor_copy(dst[:, s * SB:(s + 1) * SB], tq_ps[:])

            # ---- phase C: attention (flash-style over causal k-tiles) ----
            with tc.tile_pool(name="ppC", bufs=2, space="PSUM") as ppC:
                for s in range(NSB):
                    n_kt = 4 * (s + 1)
                    for h in range(HPC):
                        qs = qT[h][:, s * SB:(s + 1) * SB]
                        pv = ppC.tile([HD + 1, 512], F32, tag="pv", bufs=3)
                        pts = []
                        for p in range(n_kt // 2):
                            sc = ppC.tile([128, 1024], F32, tag="sc", bufs=2)
                            for half in range(2):
                                i = 2 * p + half
                                nc.tensor.matmul(
                                    sc[:, half * 512:(half + 1) * 512],
                                    kT[:, i * 128:(i + 1) * 128], qs,
                                    start=True, stop=True)
                            pt = ptp.tile([128, 1024], BF16, tag="pt")
                            nc.scalar.activation(
                                pt[:], sc[:],
                                mybir.ActivationFunctionType.Exp, scale=SCALE)
                            for half in range(2):
                                i = 2 * p + half
                                if i >= 4 * s:  # diagonal tile: causal mask
                                    nc.vector.tensor_tensor(
                                        pt[:, half * 512:(half + 1) * 512],
                                        pt[:, half * 512:(half + 1) * 512],
                                        mask_sb[:, i - 4 * s, :],
                                        mybir.AluOpType.mult)
                            if p > 0:
                                for half in range(2):
                                    i = 2 * (p - 1) + half
                                    nc.tensor.matmul(
                                        pv[:], v_sb[:, i, :],
                                        pts[p - 1][:, half * 512:(half + 1) * 512],
                                        start=(i == 0), stop=False)
                            pts.append(pt)
                        for half in range(2):
                            i = n_kt - 2 + half
                            nc.tensor.matmul(
                                pv[:], v_sb[:, i, :],
                                pts[-1][:, half * 512:(half + 1) * 512],
                                start=(i == 0), stop=(i == n_kt - 1))

                        # epilogue: out_h = pv[0:64] / (S_row + exp(sink_h))
                        srow = epp.tile([1, 512], F32, tag="srow", bufs=2)
                        nc.scalar.copy(srow[:], pv[HD:HD + 1, :])
                        nc.vector.tensor_scalar(srow[:], srow[:],
                                                es_sb[0:1, h:h + 1], None,
                                                mybir.AluOpType.add)
                        rrow = epp.tile([1, 512], F32, tag="rrow", bufs=2)
                        nc.vector.reciprocal_approx_fast(rrow[:], srow[:])
                        rbc = epp.tile([HD, 512], F32, tag="rbc")
                        nc.gpsimd.partition_broadcast(rbc[:], rrow[0:1, :])
                        nc.vector.tensor_tensor(
                            oT[h][:, s * SB:(s + 1) * SB], pv[0:HD, :], rbc[:],
                            mybir.AluOpType.mult)
                        # stream this (head, block) slice into its A2A shards
                        nc.sync.dma_start(
                            a2a_in[:].rearrange("(j hh p) n -> hh p j n",
                                                j=NCORES, hh=HPC)[h][:, 2 * s:2 * s + 2],
                            oT[h][:, s * SB:(s + 1) * SB].rearrange(
                                "p (j n) -> p j n", j=2))

            # ---- redistribute heads -> sequence slices (AllToAll) ----
            nc.gpsimd.collective_compute(
                "AllToAll", mybir.AluOpType.bypass,
                replica_groups=[list(range(NCORES))],
                ins=[a2a_in.opt()], outs=[a2a_out.opt()])
            ag_sb = cp.tile([128, ND, MYS], BF16)
            nc.sync.dma_start(ag_sb[:], a2a_out[:].rearrange(
                "(o p) n -> p o n", p=128))

            # ---- output projection for my sequence slice ----
            with tc.tile_pool(name="ppD", bufs=4, space="PSUM") as ppD:
                for m in range(MYS // 128):
                    for n in range(DIM // 512):
                        fp = ppD.tile([128, 512], F32, tag="fp", bufs=4)
                        for kt in range(ND):
                            nc.tensor.matmul(
                                fp[:], ag_sb[:, kt, m * 128:(m + 1) * 128],
                                woT_sb[:, kt, n * 512:(n + 1) * 512],
                                start=(kt == 0), stop=False)
                        nc.tensor.matmul(fp[:], ones_sb[0:1, :],
                                         wob_sb[0:1, n * 512:(n + 1) * 512],
                                         start=False, stop=True)
                        fo = fnp.tile([128, 512], F32, tag="fo")
                        if (m * 4 + n) % 2 == 0:
                            nc.scalar.copy(fo[:], fp[:])
                        else:
                            nc.vector.tensor_copy(fo[:], fp[:])
                        nc.sync.dma_start(
                            out_e[m * 128:(m + 1) * 128,
                                  n * 512:(n + 1) * 512], fo[:])

    nc.compile()
    return nc


def _host_prep(x, rope_cache, wq_w, wq_b, wk_w, wk_b, wv_w, wv_b,
               wo_w, wo_b, sinks):
    """Build the per-core input maps (sharding + layout prep)."""
    x2 = np.ascontiguousarray(np.asarray(x, np.float32).reshape(S, DIM))
    cos = np.asarray(rope_cache[:, :HD // 2], np.float32)
    sin = np.asarray(rope_cache[:, HD // 2:], np.float32)
    cosd = np.ascontiguousarray(np.concatenate([cos, cos], axis=1))
    nsin = np.ascontiguousarray(-sin)
    psin = np.ascontiguousarray(sin)
    # causal masks for the 4 diagonal 128-row k-tiles of a 512-col q block
    masks = np.zeros((4, 128, SB), np.float32)
    for d in range(4):
        for p in range(128):
            masks[d, p, d * 128 + p:] = 1.0
    masks = masks.astype(ml_dtypes.bfloat16)
    ident = np.eye(128, dtype=ml_dtypes.bfloat16)
    woT = np.ascontiguousarray(
        np.asarray(wo_w, np.float32).T.astype(ml_dtypes.bfloat16))
    wob = np.asarray(wo_b, np.float32).reshape(1, DIM)

    in_maps = []
    for c in range(NCORES):
        qsl = slice(c * QW, (c + 1) * QW)
        ksl = slice(c * HD, (c + 1) * HD)
        wqkvT = np.ascontiguousarray(np.concatenate([
            np.asarray(wq_w, np.float32)[qsl].T,
            np.asarray(wk_w, np.float32)[ksl].T,
            np.asarray(wv_w, np.float32)[ksl].T],
            axis=1).astype(ml_dtypes.bfloat16))
        qkvb = np.ascontiguousarray(np.concatenate([
            np.asarray(wq_b, np.float32)[qsl],
            np.asarray(wk_b, np.float32)[ksl],
            np.asarray(wv_b, np.float32)[ksl]])).reshape(1, QKV)
        sinks4 = np.ascontiguousarray(
            np.asarray(sinks, np.float32)[c * HPC:(c + 1) * HPC]).reshape(1, HPC)
        in_maps.append({
            "x": x2, "wqkvT": wqkvT, "qkvb": qkvb, "cosd": cosd,
            "nsin": nsin, "psin": psin, "masks": masks, "ident": ident,
            "woT": woT, "wob": wob, "sinks4": sinks4,
        })
    return in_maps


def kernel(**inputs):
    global last_exec_time_ns
    if "nc" not in _cache:
        _cache["nc"] = _build()
    nc = _cache["nc"]
    in_maps = _host_prep(**inputs)
    trace = bool(int(os.environ.get("BASS_KERNEL_TRACE", "0")))
    if trace:
        try:
            _install_ntff_shim()
        except Exception:
            trace = False
    res = run_bass_kernel_spmd(nc, in_maps, core_ids=list(range(NCORES)),
                               trace=trace)
    last_exec_time_ns = res.exec_time_ns
    out = np.concatenate([res.results[c]["out"] for c in range(NCORES)],
                         axis=0)
    return out.reshape(B, S, NH * HD)
